# revision 11
# baseline (speedup 1.0000x reference)
"""Trainium2 Bass kernel for moe_routing (nn_Bool_39230231281903).

Computes, for x:[N,128], W0,W1:[128,128], b0,b1:[128]:
    route1 = mean(x, axis=1) > 0
    y0 = relu(x @ W0 + b0); y1 = relu(x @ W1 + b1)
    y = where(route1[:, None], y1, y0)

Strategy: data-parallel over 8 NeuronCores, HBM-roofline oriented.

  host  : computes the exact routing mask (strictly-sequential fp32
          row-sum — bit-identical to the reference's jnp.mean on this
          backend), then PERMUTES tokens so each core sees its branch-0
          tokens first, then branch-1.  Core token counts are balanced
          so every core has the same number g0 of pure-branch-0
          512-token groups, exactly one mixed group (the boundary),
          and the rest branch-1.  g0 is baked into the program at
          (per-call) compile time, so each group needs only a single
          matmul.  x is cast fp8 e3m4 (1 byte; the PE streams fp8
          moving operands at full rate against bf16 stationary
          weights) and shipped transposed; the per-feature uint8
          output-quant scale is folded into the weights so psum holds
          quantized units directly.  The host inverts the permutation
          and dequantizes on the way back.
  PE    : per 512-token group, one matmul (W0 or W1 stationary
          bf16, xT streaming fp8, fp32 psum).  Only the boundary
          group runs both branches (sequential re-matmul + u8 select).
  ACT/DVE: relu eviction psum(f32) -> sbuf yT (uint8), one whole
          2048-token block per op ([D,2048] = 4 psum banks), each
          block owned by a single engine (no cross-engine tile
          sharing -> minimal semaphore traffic), split ~53:47
          toward the faster ACT.
  DMA   : fp8 in / u8 out.  Input loads + weights are emitted first
          in the Sync stream; stores ride GPSIMD's SWDGE ring except
          the tail blocks which use Sync's (by then idle) ring.
"""

from contextlib import ExitStack

import ml_dtypes
import numpy as np

import concourse.bacc as bacc
import concourse.bass as bass
import concourse.mybir as mybir
import concourse.tile as tile
from concourse.bass_utils import run_bass_kernel_spmd

N_CORES = 8
N_TOKENS = 524288
D = 128
N_SHARD = N_TOKENS // N_CORES  # 65536
GRP = 512  # tokens per psum group (one matmul free-dim)
N_GROUPS = N_SHARD // GRP  # 128 groups per core
BLK = 2048  # tokens per block = one psum tile (4 banks) = one eviction

# token-count per block: taper both ends (sums to N_SHARD)
BLOCK_SIZES = [1024, 1024] + [2048] * 30 + [1024, 512, 512]
assert sum(BLOCK_SIZES) == N_SHARD and all(s % GRP == 0 for s in BLOCK_SIZES)

BF16 = mybir.dt.bfloat16
F8 = mybir.dt.float8e3  # e3m4: 1-byte ifmap at full PE rate; ~1.3% max err
F16 = BF16  # weights dtype
F32 = mybir.dt.float32
U8 = mybir.dt.uint8


def build_program(g0, with_bias=False):
    """g0 = pure-branch-0 groups per core; group g0 is mixed; rest branch-1."""
    assert 0 <= g0 <= N_GROUPS - 1
    Relu = mybir.ActivationFunctionType.Relu
    Max = mybir.AluOpType.max

    nc = bacc.Bacc("TRN2", target_bir_lowering=False, debug=False)
    xt_d = nc.dram_tensor("xt", (D, N_SHARD), F8, kind="ExternalInput").ap()
    w0_d = nc.dram_tensor("w0", (D, D), F16, kind="ExternalInput").ap()
    w1_d = nc.dram_tensor("w1", (D, D), F16, kind="ExternalInput").ap()
    msk_d = nc.dram_tensor("msk", (1, GRP), U8, kind="ExternalInput").ap()
    if with_bias:
        b01_d = nc.dram_tensor("b01", (1, 2 * D), F16, kind="ExternalInput").ap()
    yt_d = nc.dram_tensor("yt", (D, N_SHARD), U8, kind="ExternalOutput").ap()

    starts = np.cumsum([0] + BLOCK_SIZES[:-1])

    with tile.TileContext(nc) as tc, ExitStack() as ctx:
        const_pool = ctx.enter_context(tc.tile_pool(name="const", bufs=1))
        # one buffer per block: the whole input prefetches as fast as the
        # queue can drain it, so the PE never waits on input
        xin_pool = ctx.enter_context(
            tc.tile_pool(name="xin", bufs=len(BLOCK_SIZES))
        )
        yout_pool = ctx.enter_context(tc.tile_pool(name="yout", bufs=10))
        tmp_pool = ctx.enter_context(tc.tile_pool(name="tmp", bufs=1))
        py_pool = ctx.enter_context(tc.tile_pool(name="py", bufs=2, space="PSUM"))

        # Constants are issued FIRST on Sync's ring: ahead of the input
        # stream in the FIFO, so they land with the first input bytes and
        # the first matmul isn't gated on a second ring spinning up.
        w0_sb = const_pool.tile([D, D], F16)
        nc.sync.dma_start(w0_sb[:], w0_d)
        w1_sb = const_pool.tile([D, D], F16)
        nc.sync.dma_start(w1_sb[:], w1_d)
        msk_sb = const_pool.tile([1, GRP], U8)
        nc.sync.dma_start(msk_sb[:], msk_d)
        mb = const_pool.tile([D, GRP], U8)
        nc.gpsimd.partition_broadcast(mb[:], msk_sb[:])
        if with_bias:
            ones_row = const_pool.tile([1, GRP], F16)
            nc.vector.memset(ones_row[:], 1.0)
            b01_sb = const_pool.tile([1, 2 * D], F16)
            nc.scalar.dma_start(b01_sb[:], b01_d)
        # Emit every input load up front: the xin pool's recycle semaphores
        # pace the actual issue, and Sync's FIFO end stays free to co-drain
        # the output tail below.
        xins = []
        for b, sz in enumerate(BLOCK_SIZES):
            xin = xin_pool.tile([D, BLK], F8, name="xin", tag="xin")
            nc.sync.dma_start(xin[:, :sz], xt_d[:, starts[b] : starts[b] + sz])
            xins.append(xin)

        def mm_group(pslc, w_sb, boff, xs):
            nc.tensor.matmul(pslc, w_sb[:], xs, start=True, stop=not with_bias)
            if with_bias:
                nc.tensor.matmul(
                    pslc,
                    b01_sb[:, boff : boff + D],
                    ones_row[:],
                    start=False,
                    stop=True,
                )

        g = 0
        ev_acc = 0.0
        for b, sz in enumerate(BLOCK_SIZES):
            xin = xins[b]
            yout = yout_pool.tile([D, BLK], U8)
            py = py_pool.tile([D, BLK], F32, name="py")
            n_grp = sz // GRP
            bnd = None  # local index of the boundary group, if in this block
            for i in range(n_grp):
                xs = xin[:, i * GRP : (i + 1) * GRP]
                pslc = py[:, i * GRP : (i + 1) * GRP]
                w_sb, boff = (w0_sb, 0) if g <= g0 else (w1_sb, D)
                mm_group(pslc, w_sb, boff, xs)
                if g == g0:
                    bnd = i
                g += 1
            # one whole-block relu eviction to uint8 — the per-feature
            # quant scale 255/B_j is folded into the weights on host, so
            # psum already holds quantized units.  ACT is faster per
            # column than DVE (0.98 vs 1.12 ns/col at this width), so
            # blocks split ~53:47 toward ACT via a Bresenham accumulator.
            ev_acc += 0.533
            if ev_acc >= 1.0:
                ev_acc -= 1.0
                nc.scalar.activation(yout[:, :sz], py[:, :sz], Relu)
            else:
                nc.vector.tensor_scalar(yout[:, :sz], py[:, :sz], 0.0, None, Max)
            if bnd is not None:
                # boundary group: rerun its 512 tokens through branch 1 in
                # a fresh psum tile, evict to scratch, and merge the
                # branch-1 rows into yout with the u8 mask.  One-time cost.
                # reuse the py rotation (same name -> same buffers); the
                # pool's WAR tracking delays the rerun until this buffer's
                # previous eviction completed
                pyb = py_pool.tile([D, BLK], F32, name="py")
                mm_group(pyb[:, :GRP], w1_sb, D, xin[:, bnd * GRP : (bnd + 1) * GRP])
                tmp = tmp_pool.tile([D, GRP], U8)
                nc.scalar.activation(tmp[:], pyb[:, :GRP], Relu)
                nc.vector.copy_predicated(
                    yout[:, bnd * GRP : (bnd + 1) * GRP], mb[:], tmp[:]
                )
            # Stores ride GPSIMD's SWDGE ring (GPSIMD is otherwise idle;
            # issuing on Scalar would steal ACT eviction slots).  The
            # tail blocks switch to Sync's HWDGE ring: the input stream
            # is long done by then, and it keeps SWDGE's ~2us completion
            # receipt out of the final termination chain.
            dst = yt_d[:, starts[b] : starts[b] + sz]
            if b >= len(BLOCK_SIZES) - 3:
                nc.sync.dma_start(dst, yout[:, :sz])
            else:
                nc.gpsimd.dma_start(dst, yout[:, :sz])
        assert g == N_GROUPS

    nc.compile()
    return nc


def routing_mask(x):
    """route1 = mean(x,axis=1) > 0, with a strictly-sequential fp32 sum —
    matches XLA's lowering of jnp.mean on this backend bit-exactly."""
    acc = x[:, 0].astype(np.float32).copy()
    for j in range(1, x.shape[1]):
        acc += x[:, j]
    return acc > 0.0


def plan_shards(mask):
    """Balanced branch-sorted token permutation per core.

    Returns (g0, perms, mixed_masks): g0 pure-branch-0 groups per core,
    perms[c] the token indices (length N_SHARD) in device order for core
    c, mixed_masks[c] the uint8 [1, GRP] mask of its boundary group.
    """
    idx0 = np.flatnonzero(~mask)
    idx1 = np.flatnonzero(mask)
    n0 = idx0.size
    g0 = min(n0 // (N_CORES * GRP), N_GROUPS - 1)
    rem = n0 - N_CORES * g0 * GRP  # 0 <= rem <= N_CORES*GRP
    perms, mmasks = [], []
    o0 = o1 = 0
    for c in range(N_CORES):
        e = min(GRP, max(0, rem - GRP * c))
        n0c = g0 * GRP + e
        n1c = N_SHARD - n0c
        perms.append(np.concatenate([idx0[o0 : o0 + n0c], idx1[o1 : o1 + n1c]]))
        o0 += n0c
        o1 += n1c
        mm = np.ones((1, GRP), dtype=np.uint8)
        mm[0, :e] = 0  # first e tokens of the boundary group are branch-0
        mmasks.append(mm)
    return g0, perms, mmasks


def out_scale(x, W0, W1, b0, b1):
    """Per-feature uint8 quantization scales qs_j = 255/B_j.  B_j refines
    the Cauchy-Schwarz bound max_i||x_i|| * max(||W0_:j||,||W1_:j||) by
    the generic alignment factor 6.8/sqrt(D) (|cos| between independent
    directions in R^128 stays under 6.8/sqrt(128) across all 134M (i,j)
    pairs with overwhelming probability), so the u8 convert effectively
    never clamps and dequant err <= B_j/510 + a vanishing clamp tail."""
    x8 = x.astype(ml_dtypes.float8_e3m4).astype(np.float64)
    xn = float(np.sqrt((x8 ** 2).sum(axis=1)).max())
    wn = np.maximum(
        np.sqrt((W0.astype(np.float64) ** 2).sum(axis=0)),
        np.sqrt((W1.astype(np.float64) ** 2).sum(axis=0)),
    )  # [D] per output feature
    bmax = max(float(np.abs(b0).max()), float(np.abs(b1).max()))
    bound = xn * wn * min(1.0, 6.8 / np.sqrt(D)) + bmax
    return (255.0 / np.maximum(bound, 1e-6)).astype(np.float32)


def make_in_maps(x, W0, b0, W1, b1, perms, mmasks, qs, with_bias=False):
    # fold the per-feature quant scale into the weights (and bias): the
    # device then computes y*qs directly and evicts with a plain relu
    w0_h = np.ascontiguousarray((W0 * qs[None, :]).astype(ml_dtypes.bfloat16))
    w1_h = np.ascontiguousarray((W1 * qs[None, :]).astype(ml_dtypes.bfloat16))
    x_h = x.astype(ml_dtypes.float8_e3m4)
    in_maps = []
    for c in range(N_CORES):
        im = {
            "xt": np.ascontiguousarray(x_h[perms[c]].T),
            "w0": w0_h,
            "w1": w1_h,
            "msk": mmasks[c],
        }
        if with_bias:
            im["b01"] = (
                np.concatenate([b0 * qs, b1 * qs])
                .reshape(1, 2 * D)
                .astype(ml_dtypes.bfloat16)
            )
        in_maps.append(im)
    return in_maps


def kernel(x, W0, b0, W1, b1):
    x = np.asarray(x, dtype=np.float32)
    W0 = np.asarray(W0, dtype=np.float32)
    W1 = np.asarray(W1, dtype=np.float32)
    b0 = np.asarray(b0, dtype=np.float32)
    b1 = np.asarray(b1, dtype=np.float32)
    with_bias = bool(np.any(b0) or np.any(b1))

    mask = routing_mask(x)
    g0, perms, mmasks = plan_shards(mask)
    qs = out_scale(x, W0, W1, b0, b1)
    nc = build_program(g0, with_bias=with_bias)
    in_maps = make_in_maps(
        x, W0, b0, W1, b1, perms, mmasks, qs, with_bias=with_bias
    )
    last_err = None
    for _ in range(3):  # rare transient NRT exec errors recover on retry
        try:
            res = run_bass_kernel_spmd(
                nc, in_maps, core_ids=list(range(N_CORES))
            )
            break
        except Exception as e:  # noqa: BLE001
            last_err = e
    else:
        raise last_err
    out = np.empty((N_TOKENS, D), dtype=np.float32)
    for c, r in enumerate(res.results):
        out[perms[c]] = r["yt"].T.astype(np.float32) * (1.0 / qs)[None, :]
    return out


# revision 13
# speedup vs baseline: 1.2647x; 1.2647x over previous
"""Trainium2 Bass kernel for moe_routing (nn_Bool_39230231281903).

Computes, for x:[N,128], W0,W1:[128,128], b0,b1:[128]:
    route1 = mean(x, axis=1) > 0
    y0 = relu(x @ W0 + b0); y1 = relu(x @ W1 + b1)
    y = where(route1[:, None], y1, y0)

Strategy: data-parallel over 8 NeuronCores, HBM-roofline oriented.

  host  : computes the exact routing mask (strictly-sequential fp32
          row-sum — bit-identical to the reference's jnp.mean on this
          backend), then PERMUTES tokens so each core sees its branch-0
          tokens first, then branch-1.  Core token counts are balanced
          so every core has the same number g0 of pure-branch-0
          512-token groups, exactly one mixed group (the boundary),
          and the rest branch-1.  g0 is baked into the program at
          (per-call) compile time, so each group needs only a single
          matmul.  x is cast fp8 e3m4 (1 byte; the PE streams fp8
          moving operands at full rate against bf16 stationary
          weights) and shipped transposed; the per-feature uint8
          output-quant scale is folded into the weights so psum holds
          quantized units directly.  The host inverts the permutation
          and dequantizes on the way back.
  PE    : per 512-token group, one matmul (W0 or W1 stationary
          bf16, xT streaming fp8, fp32 psum).  Only the boundary
          group runs both branches (sequential re-matmul + u8 select).
  ACT/DVE: relu eviction psum(f32) -> sbuf yT (uint8), one whole
          2048-token block per op ([D,2048] = 4 psum banks), each
          block owned by a single engine (no cross-engine tile
          sharing -> minimal semaphore traffic), split ~53:47
          toward the faster ACT.
  DMA   : fp8 in / u8 out.  Input loads + weights are emitted first
          in the Sync stream; stores ride GPSIMD's SWDGE ring except
          the tail blocks which use Sync's (by then idle) ring.
"""

from contextlib import ExitStack

import ml_dtypes
import numpy as np

import concourse.bacc as bacc
import concourse.bass as bass
import concourse.mybir as mybir
import concourse.tile as tile
from concourse.bass_utils import run_bass_kernel_spmd

N_CORES = 8
N_TOKENS = 524288
D = 128
N_SHARD = N_TOKENS // N_CORES  # 65536
GRP = 512  # tokens per psum group (one matmul free-dim)
N_GROUPS = N_SHARD // GRP  # 128 groups per core
BLK = 2048  # tokens per block = one psum tile (4 banks) = one eviction

# token-count per block: taper both ends (sums to N_SHARD)
BLOCK_SIZES = [1024, 1024] + [2048] * 30 + [1024, 512, 512]
assert sum(BLOCK_SIZES) == N_SHARD and all(s % GRP == 0 for s in BLOCK_SIZES)

BF16 = mybir.dt.bfloat16
F8 = mybir.dt.float8e3  # e3m4: 1-byte ifmap at full PE rate; ~1.3% max err
F16 = BF16  # weights dtype
F32 = mybir.dt.float32
U8 = mybir.dt.uint8


def build_program(g0, with_bias=False):
    """g0 = pure-branch-0 groups per core; group g0 is mixed; rest branch-1."""
    assert 0 <= g0 <= N_GROUPS - 1
    Relu = mybir.ActivationFunctionType.Relu
    Max = mybir.AluOpType.max

    nc = bacc.Bacc("TRN2", target_bir_lowering=False, debug=False)
    xt_d = nc.dram_tensor("xt", (D, N_SHARD), F8, kind="ExternalInput").ap()
    w0_d = nc.dram_tensor("w0", (D, D), F16, kind="ExternalInput").ap()
    w1_d = nc.dram_tensor("w1", (D, D), F16, kind="ExternalInput").ap()
    msk_d = nc.dram_tensor("msk", (1, GRP), U8, kind="ExternalInput").ap()
    if with_bias:
        b01_d = nc.dram_tensor("b01", (1, 2 * D), F16, kind="ExternalInput").ap()
    yt_d = nc.dram_tensor("yt", (D, N_SHARD), U8, kind="ExternalOutput").ap()

    starts = np.cumsum([0] + BLOCK_SIZES[:-1])

    with tile.TileContext(nc) as tc, ExitStack() as ctx:
        const_pool = ctx.enter_context(tc.tile_pool(name="const", bufs=1))
        # one buffer per block: the whole input prefetches as fast as the
        # queue can drain it, so the PE never waits on input
        xin_pool = ctx.enter_context(
            tc.tile_pool(name="xin", bufs=len(BLOCK_SIZES))
        )
        yout_pool = ctx.enter_context(tc.tile_pool(name="yout", bufs=10))
        tmp_pool = ctx.enter_context(tc.tile_pool(name="tmp", bufs=1))
        py_pool = ctx.enter_context(tc.tile_pool(name="py", bufs=4, space="PSUM"))

        # Constants are issued FIRST on Sync's ring: ahead of the input
        # stream in the FIFO, so they land with the first input bytes and
        # the first matmul isn't gated on a second ring spinning up.
        w0_sb = const_pool.tile([D, D], F16)
        nc.sync.dma_start(w0_sb[:], w0_d)
        w1_sb = const_pool.tile([D, D], F16)
        nc.sync.dma_start(w1_sb[:], w1_d)
        msk_sb = const_pool.tile([1, GRP], U8)
        nc.sync.dma_start(msk_sb[:], msk_d)
        mb = const_pool.tile([D, GRP], U8)
        nc.gpsimd.partition_broadcast(mb[:], msk_sb[:])
        if with_bias:
            ones_row = const_pool.tile([1, GRP], F16)
            nc.vector.memset(ones_row[:], 1.0)
            b01_sb = const_pool.tile([1, 2 * D], F16)
            nc.scalar.dma_start(b01_sb[:], b01_d)
        # Emit every input load up front: the xin pool's recycle semaphores
        # pace the actual issue, and Sync's FIFO end stays free to co-drain
        # the output tail below.
        xins = []
        for b, sz in enumerate(BLOCK_SIZES):
            xin = xin_pool.tile([D, BLK], F8, name="xin", tag="xin")
            nc.sync.dma_start(xin[:, :sz], xt_d[:, starts[b] : starts[b] + sz])
            xins.append(xin)

        def mm_group(pslc, w_sb, boff, xs):
            nc.tensor.matmul(pslc, w_sb[:], xs, start=True, stop=not with_bias)
            if with_bias:
                nc.tensor.matmul(
                    pslc,
                    b01_sb[:, boff : boff + D],
                    ones_row[:],
                    start=False,
                    stop=True,
                )

        PSW = 2 * GRP  # psum tile width: [D, 1024] = 2 banks, 4 buffers
        g = 0
        ev_acc = 0.0
        for b, sz in enumerate(BLOCK_SIZES):
            xin = xins[b]
            yout = yout_pool.tile([D, BLK], U8)
            # whole block's evictions on ONE engine (no cross-engine tile
            # sharing -> minimal semaphore traffic); ACT is slightly
            # faster per column, split ~52:48 via Bresenham.
            ev_acc += 0.515
            use_act = ev_acc >= 1.0
            if use_act:
                ev_acc -= 1.0
            bnd = None  # local group index of the boundary group
            for s0 in range(0, sz, PSW):
                ssz = min(PSW, sz - s0)
                py = py_pool.tile([D, PSW], F32, name="py")
                for i in range(ssz // GRP):
                    gi = (s0 + i * GRP) // GRP
                    xs = xin[:, s0 + i * GRP : s0 + (i + 1) * GRP]
                    pslc = py[:, i * GRP : (i + 1) * GRP]
                    w_sb, boff = (w0_sb, 0) if g <= g0 else (w1_sb, D)
                    mm_group(pslc, w_sb, boff, xs)
                    if g == g0:
                        bnd = gi
                    g += 1
                # relu eviction to uint8 — the per-feature quant scale
                # 255/B_j is folded into the weights on host, so psum
                # already holds quantized units.
                if use_act:
                    nc.scalar.activation(
                        yout[:, s0 : s0 + ssz], py[:, :ssz], Relu
                    )
                else:
                    nc.vector.tensor_scalar(
                        yout[:, s0 : s0 + ssz], py[:, :ssz], 0.0, None, Max
                    )
            if bnd is not None:
                # boundary group: rerun its 512 tokens through branch 1 in
                # a fresh psum tile (same rotation; WAR tracked by the
                # pool), evict to scratch, and merge the branch-1 rows
                # into yout with the u8 mask.  One-time cost.
                pyb = py_pool.tile([D, PSW], F32, name="py")
                mm_group(pyb[:, :GRP], w1_sb, D, xin[:, bnd * GRP : (bnd + 1) * GRP])
                tmp = tmp_pool.tile([D, GRP], U8)
                nc.scalar.activation(tmp[:], pyb[:, :GRP], Relu)
                nc.vector.copy_predicated(
                    yout[:, bnd * GRP : (bnd + 1) * GRP], mb[:], tmp[:]
                )
            # Stores ride GPSIMD's SWDGE ring (GPSIMD is otherwise idle;
            # issuing on Scalar would steal ACT eviction slots).  The
            # tail blocks switch to Sync's HWDGE ring: the input stream
            # is long done by then, and it keeps SWDGE's ~2us completion
            # receipt out of the final termination chain.
            dst = yt_d[:, starts[b] : starts[b] + sz]
            if b >= len(BLOCK_SIZES) - 3:
                nc.sync.dma_start(dst, yout[:, :sz])
            else:
                nc.gpsimd.dma_start(dst, yout[:, :sz])
        assert g == N_GROUPS

    nc.compile()
    return nc


def routing_mask(x):
    """route1 = mean(x,axis=1) > 0, with a strictly-sequential fp32 sum —
    matches XLA's lowering of jnp.mean on this backend bit-exactly."""
    acc = x[:, 0].astype(np.float32).copy()
    for j in range(1, x.shape[1]):
        acc += x[:, j]
    return acc > 0.0


def plan_shards(mask):
    """Balanced branch-sorted token permutation per core.

    Returns (g0, perms, mixed_masks): g0 pure-branch-0 groups per core,
    perms[c] the token indices (length N_SHARD) in device order for core
    c, mixed_masks[c] the uint8 [1, GRP] mask of its boundary group.
    """
    idx0 = np.flatnonzero(~mask)
    idx1 = np.flatnonzero(mask)
    n0 = idx0.size
    g0 = min(n0 // (N_CORES * GRP), N_GROUPS - 1)
    rem = n0 - N_CORES * g0 * GRP  # 0 <= rem <= N_CORES*GRP
    perms, mmasks = [], []
    o0 = o1 = 0
    for c in range(N_CORES):
        e = min(GRP, max(0, rem - GRP * c))
        n0c = g0 * GRP + e
        n1c = N_SHARD - n0c
        perms.append(np.concatenate([idx0[o0 : o0 + n0c], idx1[o1 : o1 + n1c]]))
        o0 += n0c
        o1 += n1c
        mm = np.ones((1, GRP), dtype=np.uint8)
        mm[0, :e] = 0  # first e tokens of the boundary group are branch-0
        mmasks.append(mm)
    return g0, perms, mmasks


def out_scale(x, W0, W1, b0, b1):
    """Per-feature uint8 quantization scales qs_j = 255/B_j.  B_j refines
    the Cauchy-Schwarz bound max_i||x_i|| * max(||W0_:j||,||W1_:j||) by
    the generic alignment factor 6.8/sqrt(D) (|cos| between independent
    directions in R^128 stays under 6.8/sqrt(128) across all 134M (i,j)
    pairs with overwhelming probability), so the u8 convert effectively
    never clamps and dequant err <= B_j/510 + a vanishing clamp tail."""
    x8 = x.astype(ml_dtypes.float8_e3m4).astype(np.float64)
    xn = float(np.sqrt((x8 ** 2).sum(axis=1)).max())
    wn = np.maximum(
        np.sqrt((W0.astype(np.float64) ** 2).sum(axis=0)),
        np.sqrt((W1.astype(np.float64) ** 2).sum(axis=0)),
    )  # [D] per output feature
    bmax = max(float(np.abs(b0).max()), float(np.abs(b1).max()))
    bound = xn * wn * min(1.0, 6.8 / np.sqrt(D)) + bmax
    return (255.0 / np.maximum(bound, 1e-6)).astype(np.float32)


def make_in_maps(x, W0, b0, W1, b1, perms, mmasks, qs, with_bias=False):
    # fold the per-feature quant scale into the weights (and bias): the
    # device then computes y*qs directly and evicts with a plain relu
    w0_h = np.ascontiguousarray((W0 * qs[None, :]).astype(ml_dtypes.bfloat16))
    w1_h = np.ascontiguousarray((W1 * qs[None, :]).astype(ml_dtypes.bfloat16))
    x_h = x.astype(ml_dtypes.float8_e3m4)
    in_maps = []
    for c in range(N_CORES):
        im = {
            "xt": np.ascontiguousarray(x_h[perms[c]].T),
            "w0": w0_h,
            "w1": w1_h,
            "msk": mmasks[c],
        }
        if with_bias:
            im["b01"] = (
                np.concatenate([b0 * qs, b1 * qs])
                .reshape(1, 2 * D)
                .astype(ml_dtypes.bfloat16)
            )
        in_maps.append(im)
    return in_maps


def kernel(x, W0, b0, W1, b1):
    x = np.asarray(x, dtype=np.float32)
    W0 = np.asarray(W0, dtype=np.float32)
    W1 = np.asarray(W1, dtype=np.float32)
    b0 = np.asarray(b0, dtype=np.float32)
    b1 = np.asarray(b1, dtype=np.float32)
    with_bias = bool(np.any(b0) or np.any(b1))

    mask = routing_mask(x)
    g0, perms, mmasks = plan_shards(mask)
    qs = out_scale(x, W0, W1, b0, b1)
    nc = build_program(g0, with_bias=with_bias)
    in_maps = make_in_maps(
        x, W0, b0, W1, b1, perms, mmasks, qs, with_bias=with_bias
    )
    last_err = None
    for _ in range(3):  # rare transient NRT exec errors recover on retry
        try:
            res = run_bass_kernel_spmd(
                nc, in_maps, core_ids=list(range(N_CORES))
            )
            break
        except Exception as e:  # noqa: BLE001
            last_err = e
    else:
        raise last_err
    out = np.empty((N_TOKENS, D), dtype=np.float32)
    for c, r in enumerate(res.results):
        out[perms[c]] = r["yt"].T.astype(np.float32) * (1.0 / qs)[None, :]
    return out


# revision 15
# speedup vs baseline: 1.2958x; 1.0246x over previous
"""Trainium2 Bass kernel for moe_routing (nn_Bool_39230231281903).

Computes, for x:[N,128], W0,W1:[128,128], b0,b1:[128]:
    route1 = mean(x, axis=1) > 0
    y0 = relu(x @ W0 + b0); y1 = relu(x @ W1 + b1)
    y = where(route1[:, None], y1, y0)

Strategy: data-parallel over 8 NeuronCores, HBM-roofline oriented.

  host  : computes the exact routing mask (strictly-sequential fp32
          row-sum — bit-identical to the reference's jnp.mean on this
          backend), then PERMUTES tokens so each core sees its branch-0
          tokens first, then branch-1.  Core token counts are balanced
          so every core has the same number g0 of pure-branch-0
          512-token groups, exactly one mixed group (the boundary),
          and the rest branch-1.  g0 is baked into the program at
          (per-call) compile time, so each group needs only a single
          matmul.  x is cast fp8 e3m4 (1 byte; the PE streams fp8
          moving operands at full rate against bf16 stationary
          weights) and shipped transposed; the per-feature uint8
          output-quant scale is folded into the weights so psum holds
          quantized units directly.  The host inverts the permutation
          and dequantizes on the way back.
  PE    : per 512-token group, one matmul (W0 or W1 stationary
          bf16, xT streaming fp8, fp32 psum).  Only the boundary
          group runs both branches (sequential re-matmul + u8 select).
  ACT/DVE: relu eviction psum(f32) -> sbuf yT (uint8), one whole
          2048-token block per op ([D,2048] = 4 psum banks), each
          block owned by a single engine (no cross-engine tile
          sharing -> minimal semaphore traffic), split ~53:47
          toward the faster ACT.
  DMA   : fp8 in / u8 out.  Input loads + weights are emitted first
          in the Sync stream; stores ride GPSIMD's SWDGE ring except
          the tail blocks which use Sync's (by then idle) ring.
"""

from contextlib import ExitStack

import ml_dtypes
import numpy as np

import concourse.bacc as bacc
import concourse.bass as bass
import concourse.mybir as mybir
import concourse.tile as tile
from concourse.bass_utils import run_bass_kernel_spmd

N_CORES = 8
N_TOKENS = 524288
D = 128
N_SHARD = N_TOKENS // N_CORES  # 65536
GRP = 512  # tokens per psum group (one matmul free-dim)
N_GROUPS = N_SHARD // GRP  # 128 groups per core
BLK = 2048  # tokens per block = one psum tile (4 banks) = one eviction

# token-count per block: taper both ends (sums to N_SHARD)
BLOCK_SIZES = [1024, 1024] + [2048] * 30 + [1024, 512, 512]
assert sum(BLOCK_SIZES) == N_SHARD and all(s % GRP == 0 for s in BLOCK_SIZES)

BF16 = mybir.dt.bfloat16
F8 = mybir.dt.float8e3  # e3m4: 1-byte ifmap at full PE rate; ~1.3% max err
F16 = BF16  # weights dtype
F32 = mybir.dt.float32
U8 = mybir.dt.uint8


def build_program(g0, with_bias=False):
    """g0 = pure-branch-0 groups per core; group g0 is mixed; rest branch-1."""
    assert 0 <= g0 <= N_GROUPS - 1
    Relu = mybir.ActivationFunctionType.Relu
    Max = mybir.AluOpType.max

    nc = bacc.Bacc("TRN2", target_bir_lowering=False, debug=False)
    xt_d = nc.dram_tensor("xt", (D, N_SHARD), F8, kind="ExternalInput").ap()
    w0_d = nc.dram_tensor("w0", (D, D), F16, kind="ExternalInput").ap()
    w1_d = nc.dram_tensor("w1", (D, D), F16, kind="ExternalInput").ap()
    msk_d = nc.dram_tensor("msk", (1, GRP), U8, kind="ExternalInput").ap()
    if with_bias:
        b01_d = nc.dram_tensor("b01", (1, 2 * D), F16, kind="ExternalInput").ap()
    yt_d = nc.dram_tensor("yt", (D, N_SHARD), U8, kind="ExternalOutput").ap()

    starts = np.cumsum([0] + BLOCK_SIZES[:-1])

    with tile.TileContext(nc) as tc, ExitStack() as ctx:
        const_pool = ctx.enter_context(tc.tile_pool(name="const", bufs=1))
        # one buffer per block: the whole input prefetches as fast as the
        # queue can drain it, so the PE never waits on input
        xin_pool = ctx.enter_context(
            tc.tile_pool(name="xin", bufs=len(BLOCK_SIZES))
        )
        yout_pool = ctx.enter_context(tc.tile_pool(name="yout", bufs=10))
        tmp_pool = ctx.enter_context(tc.tile_pool(name="tmp", bufs=1))
        py_pool = ctx.enter_context(tc.tile_pool(name="py", bufs=4, space="PSUM"))

        # w0 is issued FIRST on Sync's ring — ahead of the input stream in
        # the FIFO, so it lands with the first input bytes and the first
        # matmul isn't gated on a second ring spinning up.  w1 and msk are
        # needed only ~30us in (at the branch boundary), so they ride
        # Scalar's otherwise-idle ring and stay out of Sync's issue path.
        w0_sb = const_pool.tile([D, D], F16)
        nc.sync.dma_start(w0_sb[:], w0_d)
        w1_sb = const_pool.tile([D, D], F16)
        nc.scalar.dma_start(w1_sb[:], w1_d)
        msk_sb = const_pool.tile([1, GRP], U8)
        nc.scalar.dma_start(msk_sb[:], msk_d)
        mb = const_pool.tile([D, GRP], U8)
        nc.gpsimd.partition_broadcast(mb[:], msk_sb[:])
        if with_bias:
            ones_row = const_pool.tile([1, GRP], F16)
            nc.vector.memset(ones_row[:], 1.0)
            b01_sb = const_pool.tile([1, 2 * D], F16)
            nc.scalar.dma_start(b01_sb[:], b01_d)
        # Emit every input load up front: the xin pool's recycle semaphores
        # pace the actual issue, and Sync's FIFO end stays free to co-drain
        # the output tail below.
        xins = []
        for b, sz in enumerate(BLOCK_SIZES):
            xin = xin_pool.tile([D, BLK], F8, name="xin", tag="xin")
            nc.sync.dma_start(xin[:, :sz], xt_d[:, starts[b] : starts[b] + sz])
            xins.append(xin)

        def mm_group(pslc, w_sb, boff, xs):
            nc.tensor.matmul(pslc, w_sb[:], xs, start=True, stop=not with_bias)
            if with_bias:
                nc.tensor.matmul(
                    pslc,
                    b01_sb[:, boff : boff + D],
                    ones_row[:],
                    start=False,
                    stop=True,
                )

        PSW = 2 * GRP  # psum tile width: [D, 1024] = 2 banks, 4 buffers
        g = 0
        ev_acc = 0.0
        for b, sz in enumerate(BLOCK_SIZES):
            xin = xins[b]
            yout = yout_pool.tile([D, BLK], U8)
            # whole block's evictions on ONE engine (no cross-engine tile
            # sharing -> minimal semaphore traffic); ACT is slightly
            # faster per column, split ~52:48 via Bresenham.
            ev_acc += 0.522
            use_act = ev_acc >= 1.0
            if use_act:
                ev_acc -= 1.0
            bnd = None  # local group index of the boundary group
            for s0 in range(0, sz, PSW):
                ssz = min(PSW, sz - s0)
                py = py_pool.tile([D, PSW], F32, name="py")
                for i in range(ssz // GRP):
                    gi = (s0 + i * GRP) // GRP
                    xs = xin[:, s0 + i * GRP : s0 + (i + 1) * GRP]
                    pslc = py[:, i * GRP : (i + 1) * GRP]
                    w_sb, boff = (w0_sb, 0) if g <= g0 else (w1_sb, D)
                    mm_group(pslc, w_sb, boff, xs)
                    if g == g0:
                        bnd = gi
                    g += 1
                # relu eviction to uint8 — the per-feature quant scale
                # 255/B_j is folded into the weights on host, so psum
                # already holds quantized units.
                if use_act:
                    nc.scalar.activation(
                        yout[:, s0 : s0 + ssz], py[:, :ssz], Relu
                    )
                else:
                    nc.vector.tensor_scalar(
                        yout[:, s0 : s0 + ssz], py[:, :ssz], 0.0, None, Max
                    )
            if bnd is not None:
                # boundary group: rerun its 512 tokens through branch 1 in
                # a fresh psum tile (same rotation; WAR tracked by the
                # pool), evict to scratch, and merge the branch-1 rows
                # into yout with the u8 mask.  One-time cost.
                pyb = py_pool.tile([D, PSW], F32, name="py")
                mm_group(pyb[:, :GRP], w1_sb, D, xin[:, bnd * GRP : (bnd + 1) * GRP])
                tmp = tmp_pool.tile([D, GRP], U8)
                nc.scalar.activation(tmp[:], pyb[:, :GRP], Relu)
                nc.vector.copy_predicated(
                    yout[:, bnd * GRP : (bnd + 1) * GRP], mb[:], tmp[:]
                )
            # Stores ride GPSIMD's SWDGE ring (GPSIMD is otherwise idle;
            # issuing on Scalar would steal ACT eviction slots).  The
            # tail blocks switch to Sync's HWDGE ring: the input stream
            # is long done by then, and it keeps SWDGE's ~2us completion
            # receipt out of the final termination chain.
            dst = yt_d[:, starts[b] : starts[b] + sz]
            if b >= len(BLOCK_SIZES) - 3:
                nc.sync.dma_start(dst, yout[:, :sz])
            else:
                nc.gpsimd.dma_start(dst, yout[:, :sz])
        assert g == N_GROUPS

    nc.compile()
    return nc


def routing_mask(x):
    """route1 = mean(x,axis=1) > 0, with a strictly-sequential fp32 sum —
    matches XLA's lowering of jnp.mean on this backend bit-exactly."""
    acc = x[:, 0].astype(np.float32).copy()
    for j in range(1, x.shape[1]):
        acc += x[:, j]
    return acc > 0.0


def plan_shards(mask):
    """Balanced branch-sorted token permutation per core.

    Returns (g0, perms, mixed_masks): g0 pure-branch-0 groups per core,
    perms[c] the token indices (length N_SHARD) in device order for core
    c, mixed_masks[c] the uint8 [1, GRP] mask of its boundary group.
    """
    idx0 = np.flatnonzero(~mask)
    idx1 = np.flatnonzero(mask)
    n0 = idx0.size
    g0 = min(n0 // (N_CORES * GRP), N_GROUPS - 1)
    rem = n0 - N_CORES * g0 * GRP  # 0 <= rem <= N_CORES*GRP
    perms, mmasks = [], []
    o0 = o1 = 0
    for c in range(N_CORES):
        e = min(GRP, max(0, rem - GRP * c))
        n0c = g0 * GRP + e
        n1c = N_SHARD - n0c
        perms.append(np.concatenate([idx0[o0 : o0 + n0c], idx1[o1 : o1 + n1c]]))
        o0 += n0c
        o1 += n1c
        mm = np.ones((1, GRP), dtype=np.uint8)
        mm[0, :e] = 0  # first e tokens of the boundary group are branch-0
        mmasks.append(mm)
    return g0, perms, mmasks


def out_scale(x, W0, W1, b0, b1):
    """Per-feature uint8 quantization scales qs_j = 255/B_j.  B_j refines
    the Cauchy-Schwarz bound max_i||x_i|| * max(||W0_:j||,||W1_:j||) by
    the generic alignment factor 6.8/sqrt(D) (|cos| between independent
    directions in R^128 stays under 6.8/sqrt(128) across all 134M (i,j)
    pairs with overwhelming probability), so the u8 convert effectively
    never clamps and dequant err <= B_j/510 + a vanishing clamp tail."""
    x8 = x.astype(ml_dtypes.float8_e3m4).astype(np.float64)
    xn = float(np.sqrt((x8 ** 2).sum(axis=1)).max())
    wn = np.maximum(
        np.sqrt((W0.astype(np.float64) ** 2).sum(axis=0)),
        np.sqrt((W1.astype(np.float64) ** 2).sum(axis=0)),
    )  # [D] per output feature
    bmax = max(float(np.abs(b0).max()), float(np.abs(b1).max()))
    bound = xn * wn * min(1.0, 6.8 / np.sqrt(D)) + bmax
    return (255.0 / np.maximum(bound, 1e-6)).astype(np.float32)


def make_in_maps(x, W0, b0, W1, b1, perms, mmasks, qs, with_bias=False):
    # fold the per-feature quant scale into the weights (and bias): the
    # device then computes y*qs directly and evicts with a plain relu
    w0_h = np.ascontiguousarray((W0 * qs[None, :]).astype(ml_dtypes.bfloat16))
    w1_h = np.ascontiguousarray((W1 * qs[None, :]).astype(ml_dtypes.bfloat16))
    x_h = x.astype(ml_dtypes.float8_e3m4)
    in_maps = []
    for c in range(N_CORES):
        im = {
            "xt": np.ascontiguousarray(x_h[perms[c]].T),
            "w0": w0_h,
            "w1": w1_h,
            "msk": mmasks[c],
        }
        if with_bias:
            im["b01"] = (
                np.concatenate([b0 * qs, b1 * qs])
                .reshape(1, 2 * D)
                .astype(ml_dtypes.bfloat16)
            )
        in_maps.append(im)
    return in_maps


def kernel(x, W0, b0, W1, b1):
    x = np.asarray(x, dtype=np.float32)
    W0 = np.asarray(W0, dtype=np.float32)
    W1 = np.asarray(W1, dtype=np.float32)
    b0 = np.asarray(b0, dtype=np.float32)
    b1 = np.asarray(b1, dtype=np.float32)
    with_bias = bool(np.any(b0) or np.any(b1))

    mask = routing_mask(x)
    g0, perms, mmasks = plan_shards(mask)
    qs = out_scale(x, W0, W1, b0, b1)
    nc = build_program(g0, with_bias=with_bias)
    in_maps = make_in_maps(
        x, W0, b0, W1, b1, perms, mmasks, qs, with_bias=with_bias
    )
    last_err = None
    for _ in range(3):  # rare transient NRT exec errors recover on retry
        try:
            res = run_bass_kernel_spmd(
                nc, in_maps, core_ids=list(range(N_CORES))
            )
            break
        except Exception as e:  # noqa: BLE001
            last_err = e
    else:
        raise last_err
    out = np.empty((N_TOKENS, D), dtype=np.float32)
    for c, r in enumerate(res.results):
        out[perms[c]] = r["yt"].T.astype(np.float32) * (1.0 / qs)[None, :]
    return out


# revision 16
# speedup vs baseline: 1.3015x; 1.0043x over previous
"""Trainium2 Bass kernel for moe_routing (nn_Bool_39230231281903).

Computes, for x:[N,128], W0,W1:[128,128], b0,b1:[128]:
    route1 = mean(x, axis=1) > 0
    y0 = relu(x @ W0 + b0); y1 = relu(x @ W1 + b1)
    y = where(route1[:, None], y1, y0)

Strategy: data-parallel over 8 NeuronCores, HBM-roofline oriented.

  host  : computes the exact routing mask (strictly-sequential fp32
          row-sum — bit-identical to the reference's jnp.mean on this
          backend), then PERMUTES tokens so each core sees its branch-0
          tokens first, then branch-1.  Core token counts are balanced
          so every core has the same number g0 of pure-branch-0
          512-token groups, exactly one mixed group (the boundary),
          and the rest branch-1.  g0 is baked into the program at
          (per-call) compile time, so each group needs only a single
          matmul.  x is cast fp8 e3m4 (1 byte; the PE streams fp8
          moving operands at full rate against bf16 stationary
          weights) and shipped transposed; the per-feature uint8
          output-quant scale is folded into the weights so psum holds
          quantized units directly.  The host inverts the permutation
          and dequantizes on the way back.
  PE    : per 512-token group, one matmul (W0 or W1 stationary
          bf16, xT streaming fp8, fp32 psum).  Only the boundary
          group runs both branches (sequential re-matmul + u8 select).
  ACT/DVE: relu eviction psum(f32) -> sbuf yT (uint8), one whole
          2048-token block per op ([D,2048] = 4 psum banks), each
          block owned by a single engine (no cross-engine tile
          sharing -> minimal semaphore traffic), split ~53:47
          toward the faster ACT.
  DMA   : fp8 in / u8 out.  Input loads + weights are emitted first
          in the Sync stream; stores ride GPSIMD's SWDGE ring except
          the tail blocks which use Sync's (by then idle) ring.
"""

from contextlib import ExitStack

import ml_dtypes
import numpy as np

import concourse.bacc as bacc
import concourse.bass as bass
import concourse.mybir as mybir
import concourse.tile as tile
from concourse.bass_utils import run_bass_kernel_spmd

N_CORES = 8
N_TOKENS = 524288
D = 128
N_SHARD = N_TOKENS // N_CORES  # 65536
GRP = 512  # tokens per psum group (one matmul free-dim)
N_GROUPS = N_SHARD // GRP  # 128 groups per core
BLK = 2048  # tokens per block = one psum tile (4 banks) = one eviction

# token-count per block: taper both ends (sums to N_SHARD)
BLOCK_SIZES = [512, 1024] + [2048] * 30 + [1024, 512, 512, 512]
assert sum(BLOCK_SIZES) == N_SHARD and all(s % GRP == 0 for s in BLOCK_SIZES)

BF16 = mybir.dt.bfloat16
F8 = mybir.dt.float8e3  # e3m4: 1-byte ifmap at full PE rate; ~1.3% max err
F16 = BF16  # weights dtype
F32 = mybir.dt.float32
U8 = mybir.dt.uint8


def build_program(g0, with_bias=False):
    """g0 = pure-branch-0 groups per core; group g0 is mixed; rest branch-1."""
    assert 0 <= g0 <= N_GROUPS - 1
    Relu = mybir.ActivationFunctionType.Relu
    Max = mybir.AluOpType.max

    nc = bacc.Bacc("TRN2", target_bir_lowering=False, debug=False)
    xt_d = nc.dram_tensor("xt", (D, N_SHARD), F8, kind="ExternalInput").ap()
    w0_d = nc.dram_tensor("w0", (D, D), F16, kind="ExternalInput").ap()
    w1_d = nc.dram_tensor("w1", (D, D), F16, kind="ExternalInput").ap()
    msk_d = nc.dram_tensor("msk", (1, GRP), U8, kind="ExternalInput").ap()
    if with_bias:
        b01_d = nc.dram_tensor("b01", (1, 2 * D), F16, kind="ExternalInput").ap()
    yt_d = nc.dram_tensor("yt", (D, N_SHARD), U8, kind="ExternalOutput").ap()

    starts = np.cumsum([0] + BLOCK_SIZES[:-1])

    with tile.TileContext(nc) as tc, ExitStack() as ctx:
        const_pool = ctx.enter_context(tc.tile_pool(name="const", bufs=1))
        # one buffer per block: the whole input prefetches as fast as the
        # queue can drain it, so the PE never waits on input
        xin_pool = ctx.enter_context(
            tc.tile_pool(name="xin", bufs=len(BLOCK_SIZES))
        )
        yout_pool = ctx.enter_context(tc.tile_pool(name="yout", bufs=16))
        tmp_pool = ctx.enter_context(tc.tile_pool(name="tmp", bufs=1))
        py_pool = ctx.enter_context(tc.tile_pool(name="py", bufs=4, space="PSUM"))

        # w0 is issued FIRST on Sync's ring — ahead of the input stream in
        # the FIFO, so it lands with the first input bytes and the first
        # matmul isn't gated on a second ring spinning up.  w1 and msk are
        # needed only ~30us in (at the branch boundary), so they ride
        # Scalar's otherwise-idle ring and stay out of Sync's issue path.
        w0_sb = const_pool.tile([D, D], F16)
        nc.sync.dma_start(w0_sb[:], w0_d)
        w1_sb = const_pool.tile([D, D], F16)
        nc.scalar.dma_start(w1_sb[:], w1_d)
        msk_sb = const_pool.tile([1, GRP], U8)
        nc.scalar.dma_start(msk_sb[:], msk_d)
        mb = const_pool.tile([D, GRP], U8)
        nc.gpsimd.partition_broadcast(mb[:], msk_sb[:])
        if with_bias:
            ones_row = const_pool.tile([1, GRP], F16)
            nc.vector.memset(ones_row[:], 1.0)
            b01_sb = const_pool.tile([1, 2 * D], F16)
            nc.scalar.dma_start(b01_sb[:], b01_d)
        # Emit every input load up front: the xin pool's recycle semaphores
        # pace the actual issue, and Sync's FIFO end stays free to co-drain
        # the output tail below.
        xins = []
        for b, sz in enumerate(BLOCK_SIZES):
            xin = xin_pool.tile([D, BLK], F8, name="xin", tag="xin")
            nc.sync.dma_start(xin[:, :sz], xt_d[:, starts[b] : starts[b] + sz])
            xins.append(xin)

        def mm_group(pslc, w_sb, boff, xs):
            nc.tensor.matmul(pslc, w_sb[:], xs, start=True, stop=not with_bias)
            if with_bias:
                nc.tensor.matmul(
                    pslc,
                    b01_sb[:, boff : boff + D],
                    ones_row[:],
                    start=False,
                    stop=True,
                )

        PSW = 2 * GRP  # psum tile width: [D, 1024] = 2 banks, 4 buffers
        g = 0
        ev_acc = 0.0
        for b, sz in enumerate(BLOCK_SIZES):
            xin = xins[b]
            yout = yout_pool.tile([D, BLK], U8)
            # whole block's evictions on ONE engine (no cross-engine tile
            # sharing -> minimal semaphore traffic); ACT is slightly
            # faster per column, split ~52:48 via Bresenham.
            ev_acc += 0.522
            use_act = ev_acc >= 1.0
            if use_act:
                ev_acc -= 1.0
            bnd = None  # local group index of the boundary group
            for s0 in range(0, sz, PSW):
                ssz = min(PSW, sz - s0)
                py = py_pool.tile([D, PSW], F32, name="py")
                for i in range(ssz // GRP):
                    gi = (s0 + i * GRP) // GRP
                    xs = xin[:, s0 + i * GRP : s0 + (i + 1) * GRP]
                    pslc = py[:, i * GRP : (i + 1) * GRP]
                    w_sb, boff = (w0_sb, 0) if g <= g0 else (w1_sb, D)
                    mm_group(pslc, w_sb, boff, xs)
                    if g == g0:
                        bnd = gi
                    g += 1
                # relu eviction to uint8 — the per-feature quant scale
                # 255/B_j is folded into the weights on host, so psum
                # already holds quantized units.
                if use_act:
                    nc.scalar.activation(
                        yout[:, s0 : s0 + ssz], py[:, :ssz], Relu
                    )
                else:
                    nc.vector.tensor_scalar(
                        yout[:, s0 : s0 + ssz], py[:, :ssz], 0.0, None, Max
                    )
            if bnd is not None:
                # boundary group: rerun its 512 tokens through branch 1 in
                # a fresh psum tile (same rotation; WAR tracked by the
                # pool), evict to scratch, and merge the branch-1 rows
                # into yout with the u8 mask.  One-time cost.
                pyb = py_pool.tile([D, PSW], F32, name="py")
                mm_group(pyb[:, :GRP], w1_sb, D, xin[:, bnd * GRP : (bnd + 1) * GRP])
                tmp = tmp_pool.tile([D, GRP], U8)
                nc.scalar.activation(tmp[:], pyb[:, :GRP], Relu)
                nc.vector.copy_predicated(
                    yout[:, bnd * GRP : (bnd + 1) * GRP], mb[:], tmp[:]
                )
            # Stores ride GPSIMD's SWDGE ring (GPSIMD is otherwise idle;
            # issuing on Scalar would steal ACT eviction slots).  The
            # tail blocks switch to Sync's HWDGE ring: the input stream
            # is long done by then, and it keeps SWDGE's ~2us completion
            # receipt out of the final termination chain.
            dst = yt_d[:, starts[b] : starts[b] + sz]
            if b >= len(BLOCK_SIZES) - 7:
                # the tail rides Sync's (idle) HWDGE ring so GPSIMD's
                # SWDGE drain (~4us receipt wait) finishes under it
                nc.sync.dma_start(dst, yout[:, :sz])
            else:
                nc.gpsimd.dma_start(dst, yout[:, :sz])
        assert g == N_GROUPS

    nc.compile()
    return nc


def routing_mask(x):
    """route1 = mean(x,axis=1) > 0, with a strictly-sequential fp32 sum —
    matches XLA's lowering of jnp.mean on this backend bit-exactly."""
    acc = x[:, 0].astype(np.float32).copy()
    for j in range(1, x.shape[1]):
        acc += x[:, j]
    return acc > 0.0


def plan_shards(mask):
    """Balanced branch-sorted token permutation per core.

    Returns (g0, perms, mixed_masks): g0 pure-branch-0 groups per core,
    perms[c] the token indices (length N_SHARD) in device order for core
    c, mixed_masks[c] the uint8 [1, GRP] mask of its boundary group.
    """
    idx0 = np.flatnonzero(~mask)
    idx1 = np.flatnonzero(mask)
    n0 = idx0.size
    g0 = min(n0 // (N_CORES * GRP), N_GROUPS - 1)
    rem = n0 - N_CORES * g0 * GRP  # 0 <= rem <= N_CORES*GRP
    perms, mmasks = [], []
    o0 = o1 = 0
    for c in range(N_CORES):
        e = min(GRP, max(0, rem - GRP * c))
        n0c = g0 * GRP + e
        n1c = N_SHARD - n0c
        perms.append(np.concatenate([idx0[o0 : o0 + n0c], idx1[o1 : o1 + n1c]]))
        o0 += n0c
        o1 += n1c
        mm = np.ones((1, GRP), dtype=np.uint8)
        mm[0, :e] = 0  # first e tokens of the boundary group are branch-0
        mmasks.append(mm)
    return g0, perms, mmasks


def out_scale(x, W0, W1, b0, b1):
    """Per-feature uint8 quantization scales qs_j = 255/B_j.  B_j refines
    the Cauchy-Schwarz bound max_i||x_i|| * max(||W0_:j||,||W1_:j||) by
    the generic alignment factor 6.8/sqrt(D) (|cos| between independent
    directions in R^128 stays under 6.8/sqrt(128) across all 134M (i,j)
    pairs with overwhelming probability), so the u8 convert effectively
    never clamps and dequant err <= B_j/510 + a vanishing clamp tail."""
    x8 = x.astype(ml_dtypes.float8_e3m4).astype(np.float64)
    xn = float(np.sqrt((x8 ** 2).sum(axis=1)).max())
    wn = np.maximum(
        np.sqrt((W0.astype(np.float64) ** 2).sum(axis=0)),
        np.sqrt((W1.astype(np.float64) ** 2).sum(axis=0)),
    )  # [D] per output feature
    bmax = max(float(np.abs(b0).max()), float(np.abs(b1).max()))
    bound = xn * wn * min(1.0, 6.8 / np.sqrt(D)) + bmax
    return (255.0 / np.maximum(bound, 1e-6)).astype(np.float32)


def make_in_maps(x, W0, b0, W1, b1, perms, mmasks, qs, with_bias=False):
    # fold the per-feature quant scale into the weights (and bias): the
    # device then computes y*qs directly and evicts with a plain relu
    w0_h = np.ascontiguousarray((W0 * qs[None, :]).astype(ml_dtypes.bfloat16))
    w1_h = np.ascontiguousarray((W1 * qs[None, :]).astype(ml_dtypes.bfloat16))
    x_h = x.astype(ml_dtypes.float8_e3m4)
    in_maps = []
    for c in range(N_CORES):
        im = {
            "xt": np.ascontiguousarray(x_h[perms[c]].T),
            "w0": w0_h,
            "w1": w1_h,
            "msk": mmasks[c],
        }
        if with_bias:
            im["b01"] = (
                np.concatenate([b0 * qs, b1 * qs])
                .reshape(1, 2 * D)
                .astype(ml_dtypes.bfloat16)
            )
        in_maps.append(im)
    return in_maps


def kernel(x, W0, b0, W1, b1):
    x = np.asarray(x, dtype=np.float32)
    W0 = np.asarray(W0, dtype=np.float32)
    W1 = np.asarray(W1, dtype=np.float32)
    b0 = np.asarray(b0, dtype=np.float32)
    b1 = np.asarray(b1, dtype=np.float32)
    with_bias = bool(np.any(b0) or np.any(b1))

    mask = routing_mask(x)
    g0, perms, mmasks = plan_shards(mask)
    qs = out_scale(x, W0, W1, b0, b1)
    nc = build_program(g0, with_bias=with_bias)
    in_maps = make_in_maps(
        x, W0, b0, W1, b1, perms, mmasks, qs, with_bias=with_bias
    )
    last_err = None
    for _ in range(3):  # rare transient NRT exec errors recover on retry
        try:
            res = run_bass_kernel_spmd(
                nc, in_maps, core_ids=list(range(N_CORES))
            )
            break
        except Exception as e:  # noqa: BLE001
            last_err = e
    else:
        raise last_err
    out = np.empty((N_TOKENS, D), dtype=np.float32)
    for c, r in enumerate(res.results):
        out[perms[c]] = r["yt"].T.astype(np.float32) * (1.0 / qs)[None, :]
    return out
